# revision 77
# baseline (speedup 1.0000x reference)
"""BertSelfAttention (B=4, S=2048, D=1024, H=16, hd=64) on 8 trn2 NeuronCores.

Sharding: core = 2*b + half. Each core handles batch b = core//2 and 8 of the
16 heads (feature slice half*512 .. half*512+512). Fully embarrassingly
parallel: no collectives.

Per-core kernel (bf16 operands, fp32 PSUM accumulation):
  Pass A: K (f-tile 0 only), V (all), Q (f-tile 0) projections from
    X^T [1024, 2048] streamed in 512-col chunks (inputs pre-rounded to bf16
    on the host; weight f-tiles streamed per head-pair).
    Q^T, K^T in [f, s] layout (head dim on partitions); V in [s, f] layout
    with a ones column per head so the PV matmul also accumulates the
    softmax denominator in PSUM row 64.
  Attention per head-pair p, per q-quarter qq (512 wide):
    S^T chunks for both heads land in one [128, 1024] PSUM tile (head A in
    cols 0:512 via PE row-group 0-63, head B in cols 512:1024 via row-group
    64-127). The exp of the scores is split across two engines so neither
    is the wall (ScalarE alone is ~1.15us per [128,1024] chunk = 293us for
    all 256 chunks):
      - ScalarE chunks: exact exp activation (mask as per-partition bias,
        1/sqrt(64) scale folded), bf16 out.
      - DVE chunks: Schraudolph exp - one tensor_scalar computing
        round(23.08312*s + (184.665*mask + 16250)) into int16, which IS the
        bf16 bit pattern of ~exp(s/8 + mask). The constant-bias ratio error
        cancels exactly in the softmax ratio; remaining +-3% sawtooth keeps
        final rel err ~1e-2 (gate 2e-2).
    PV accumulates ctx^T (rows 0..63) + denominator (row 64) over the 16
    k-chunks. Finalize: stage ctx out of PSUM (fast DVE copy, keeps the PE
    fed), reciprocal_approx_fast of the denominator row (5x faster than the
    iterative-divide reciprocal), gpsimd partition-broadcast + multiply +
    bias add (keeps DVE free for exp), DMA out.
  K/Q projections for pair p+1 are emitted so they execute under attention
  of pair p (X^T re-streamed per pair) - keeps the PE dense so the HAM
  clock gate stays open.
"""

import numpy as np
from ml_dtypes import bfloat16 as _bf16np

S = 2048  # sequence length
DM = 1024  # model dim
F = 512  # features per core (8 heads x 64)
HL = 8  # heads per core
HD = 64  # head dim
NC = 8  # cores

# k-chunks (of 16 per q-quarter) whose exp runs on DVE via the Schraudolph
# bit trick instead of ScalarE's exact exp. Placed at the odd slot of a
# chunk-pair so most pairs are (ScalarE, DVE) and the two engines overlap.
# Quarters where the DVE also carries finalize work get fewer trick chunks
# so the in-order DVE queue never stalls the PE's PV matmuls.
def dve_chunks(p, qq):
    return frozenset((1, 3, 5, 9, 11, 13))
# bf16 bit pattern of exp(y) ~= 184.66496*y + 16250 (128/ln2; bias tuned so
# values center in range - any constant offset cancels in the softmax ratio)
EXP_MUL = 184.66496 * 0.125  # scores arrive unscaled; fold the 1/sqrt(64)
EXP_BIAS = 16250.0


def build_nc():
    import concourse.bass as bass
    import concourse.mybir as mybir
    import concourse.tile as tile
    from concourse import bacc
    from concourse.bass import ds, ts

    f32 = mybir.dt.float32
    f32r = mybir.dt.float32r
    bf16 = mybir.dt.bfloat16
    i16 = mybir.dt.int16
    EXP = mybir.ActivationFunctionType.Exp
    PSUM = bass.MemorySpace.PSUM

    nc = bacc.Bacc("TRN2", target_bir_lowering=False, debug=False, num_devices=NC)

    x_d = nc.declare_dram_parameter("x_t", [4 * DM, 512], bf16, isOutput=False)
    bqr_d = nc.declare_dram_parameter("bqr", [1, F], bf16, isOutput=False)
    wq_d = nc.declare_dram_parameter("wq_t", [4 * DM, 128], bf16, isOutput=False)
    wk_d = nc.declare_dram_parameter("wk_t", [4 * DM, 128], bf16, isOutput=False)
    wv_d = nc.declare_dram_parameter("wv_t", [DM, F], bf16, isOutput=False)
    bv_d = nc.declare_dram_parameter("bv", [F, 1], f32, isOutput=False)
    mask_d = nc.declare_dram_parameter("mask", [128, 16], f32, isOutput=False)
    out_d = nc.declare_dram_parameter("out_t", [F, S], f32, isOutput=True)

    mm = nc.tensor.matmul

    with tile.TileContext(nc) as tc:
        with (
            tc.tile_pool(name="const", bufs=1) as const,
            tc.tile_pool(name="w", bufs=1) as wpool,
            tc.tile_pool(name="wqk", bufs=4) as wqkp,
            tc.tile_pool(name="qkv", bufs=1) as qkv,
            tc.tile_pool(name="pqkv", bufs=2, space=PSUM) as pqkv,
            tc.tile_pool(name="s_ps", bufs=2, space=PSUM) as sp,
            tc.tile_pool(name="ctxA", bufs=1, space=PSUM) as cpA,
            tc.tile_pool(name="ctxB", bufs=1, space=PSUM) as cpB,
            tc.tile_pool(name="expp", bufs=12) as ep,
            tc.tile_pool(name="stagep", bufs=14) as stp,
            tc.tile_pool(name="gatp", bufs=2) as gp,
            tc.tile_pool(name="fin", bufs=4) as fp,
        ):
            # critical-path first: wk/wq f-tile 0 and x chunk 0 on the sync
            # queue before anything else touches DMA
            def load_w_tile(w_d, i):
                # per-c DMAs so the first proj matmul only waits for 64KB
                wt = wqkp.tile([128, 8, 128], bf16, tag="wt")
                for c in range(8):
                    nc.sync.dma_start(wt[:, c, :], w_d[ds(i * DM + c * 128, 128), :])
                return wt

            # X^T resident in SBUF for the whole kernel (32KB/partition):
            # x_sb[p, c, n, s'] = X^T[c*128+p, n*512+s']
            x_sb = qkv.tile([128, 8, 4, 512], bf16)

            def x_load(n, eng):
                # one strided DMA per chunk (descriptor issue costs ~0.7us -
                # per-c splitting jams the issuing queue), spread over
                # different engines' DMA queues so the ~15us transfers of
                # the four chunks run in parallel
                eng.dma_start(
                    x_sb[:, :, n, :],
                    x_d[ds(n * DM, DM), :].rearrange("(c p) s -> p c s", p=128),
                )

            # first chunk per-c so the very first projection matmul only
            # waits on one 64KB transfer, not the whole 512KB chunk
            for c in range(8):
                nc.gpsimd.dma_start(x_sb[:, c, 0, :], x_d[ds(c * 128, 128), :])
            # chunk 1 split across two queues - K(0,1) needs it ~6us after
            # pass A starts, a full-chunk transfer on one queue arrives late
            nc.scalar.dma_start(
                x_sb[:, 0:4, 1, :],
                x_d[ds(DM, 512), :].rearrange("(c p) s -> p c s", p=128),
            )
            nc.gpsimd.dma_start(
                x_sb[:, 4:8, 1, :],
                x_d[ds(DM + 512, 512), :].rearrange("(c p) s -> p c s", p=128),
            )
            x_load(2, nc.gpsimd)
            wkt = load_w_tile(wk_d, 0)
            wqt = load_w_tile(wq_d, 0)
            x_load(3, nc.sync)  # after the weight tiles on the sync queue

            # memset can't emit float32r directly; memset f32 then round-copy
            ones_f32 = const.tile([128, 128], f32)
            nc.vector.memset(ones_f32[:], 1.0)
            warm = const.tile([1, 1], f32)
            nc.scalar.activation(warm[:], ones_f32[0:1, 0:1], EXP)
            # warm the PE during the initial DMA wait: ~26 dummy matmuls keep
            # the HAM activity window busy so the first real matmuls run at
            # 2.4GHz instead of paying the 1.2GHz cold penalty
            warm_bf = const.tile([128, 512], bf16)
            nc.vector.memset(warm_bf[:], 1.0)
            warm_ps = pqkv.tile([128, 512], f32, tag="pqkv")
            for i in range(26):
                mm(
                    warm_ps[:],
                    warm_bf[:, 0:128],
                    warm_bf[:],
                    start=(i == 0),
                    stop=(i == 25),
                )
            wv_sb = wpool.tile([128, 8, F], bf16)
            for c in range(8):
                nc.gpsimd.dma_start(wv_sb[:, c, :], wv_d[ts(c, 128), :])

            # Q bias as a [1, 512] row (host-prepared) for the rank-1 PE
            # accumulation into the projection PSUM; K bias is dropped - it
            # only shifts scores by a per-query constant, which softmax
            # cancels exactly. A ones row is the rank-1 matmul's rhs.
            bqr_sb = const.tile([1, F], bf16)
            nc.gpsimd.dma_start(bqr_sb[:], bqr_d[:])
            ones_row = const.tile([1, 512], bf16)
            nc.vector.memset(ones_row[:], 1.0)
            bv_sb = const.tile([128, 4], f32)
            for i in range(4):
                nc.gpsimd.dma_start(bv_sb[:, i : i + 1], bv_d[ts(i, 128), :])
            mask_sb = const.tile([128, 16], f32)
            nc.gpsimd.dma_start(mask_sb[:], mask_d[:])
            # per-(k-partition, chunk) additive constant for the DVE exp trick
            c16_sb = const.tile([128, 16], f32)
            nc.vector.tensor_scalar(
                out=c16_sb[:],
                in0=mask_sb[:],
                scalar1=184.66496,
                scalar2=EXP_BIAS,
                op0=mybir.AluOpType.mult,
                op1=mybir.AluOpType.add,
            )

            # Q^T / K^T: [f, s] layout as 4 partition tiles of 128 features.
            q_sb = qkv.tile([128, 4, S], bf16)
            k_sb = qkv.tile([128, 4, S], bf16)
            # V in [k, head, d+1] layout; column 64 = 1.0 (denominator trick).
            v_sb = qkv.tile([128, 16, HL, HD + 1], bf16)
            nc.vector.tensor_copy(
                v_sb[:, :, :, HD], ones_f32[:, 0:128].rearrange("p (a b) -> p a b", a=16)
            )

            def qk_proj(wt, dst, i, n, with_bias=False):
                ps = pqkv.tile([128, 512], f32, tag="pqkv")
                for c in range(8):
                    mm(
                        ps[:],
                        wt[:, c, :],
                        x_sb[:, c, n, :],
                        start=(c == 0),
                        stop=(c == 7) and not with_bias,
                    )
                if with_bias:
                    # bias as rank-1 accumulation: ps += bq[f-tile] x ones
                    mm(
                        ps[:],
                        bqr_sb[0:1, ts(i, 128)],
                        ones_row[:],
                        start=False,
                        stop=True,
                    )
                # plain copy -> ScalarE, so the next pair's QK matmuls never
                # depend on the congested DVE queue
                nc.scalar.copy(dst[:, i, ts(n, 512)], ps[:])

            def v_proj(m, n, on_scalar=False):
                kc = n * 4 + m
                ps = pqkv.tile([128, 512], f32, tag="pqkv")
                for c in range(8):
                    mm(
                        ps[:],
                        x_sb[:, c, n, ts(m, 128)],
                        wv_sb[:, c, :],
                        start=(c == 0),
                        stop=(c == 7),
                    )
                dst = v_sb[:, kc, :, 0:HD]
                src = ps[:].rearrange("p (h d) -> p h d", h=HL)
                if on_scalar:
                    # keeps the DVE queue free for the exp trick when V is
                    # staged just-in-time inside attn(0,0)
                    nc.scalar.copy(dst, src)
                else:
                    nc.vector.tensor_copy(dst, src)

            def attn_pair(p, qq):
                hA, hB = 2 * p, 2 * p + 1
                qsl = ds(qq * 512, 512)
                ctxA = cpA.tile([HD + 1, 512], f32, tag="cA")
                ctxB = cpB.tile([HD + 1, 512], f32, tag="cB")
                def emit_qk(c):
                    sps = sp.tile([128, 1024], f32, tag="s")
                    mm(
                        sps[:, 0:512],
                        k_sb[0:64, p, ds(c * 128, 128)],
                        q_sb[0:64, p, qsl],
                        start=True,
                        stop=True,
                        tile_position=(0, 0),
                    )
                    mm(
                        sps[:, 512:1024],
                        k_sb[64:128, p, ds(c * 128, 128)],
                        q_sb[64:128, p, qsl],
                        start=True,
                        stop=True,
                        tile_position=(64, 0),
                    )
                    return sps

                dchunks = dve_chunks(p, qq)

                def emit_exp(c, sps):
                    et = ep.tile([128, 1024], bf16, tag="e")
                    if c in dchunks:
                        nc.vector.tensor_scalar(
                            out=et[:].bitcast(i16),
                            in0=sps[:],
                            scalar1=EXP_MUL,
                            scalar2=c16_sb[:, c : c + 1],
                            op0=mybir.AluOpType.mult,
                            op1=mybir.AluOpType.add,
                        )
                    else:
                        nc.scalar.activation(
                            et[:], sps[:], EXP, bias=mask_sb[:, c : c + 1], scale=0.125
                        )
                    return et

                def emit_pv(c, et):
                    mm(
                        ctxA[:],
                        v_sb[:, c, hA, :],
                        et[:, 0:512],
                        start=(c == 0),
                        stop=(c == 15),
                    )
                    mm(
                        ctxB[:],
                        v_sb[:, c, hB, :],
                        et[:, 512:1024],
                        start=(c == 0),
                        stop=(c == 15),
                    )

                # software-pipelined emission in chunk pairs: the PE queue
                # runs [PV(2t-2) PV(2t-1) QK(2t+2) QK(2t+3)] while both exp
                # engines process chunks 2t/2t+1 concurrently - the serial
                # chain exp(c)->PV(c)->QK(c+1)->exp(c+1) is broken.
                ets = {}
                for t in range(8):
                    s0 = emit_qk(2 * t)
                    s1 = emit_qk(2 * t + 1)
                    ets[2 * t] = emit_exp(2 * t, s0)
                    ets[2 * t + 1] = emit_exp(2 * t + 1, s1)
                    if t > 0:
                        emit_pv(2 * t - 2, ets.pop(2 * t - 2))
                        emit_pv(2 * t - 1, ets.pop(2 * t - 1))
                emit_pv(14, ets.pop(14))
                emit_pv(15, ets.pop(15))
                # finalize inline per head, exactly like the proven baseline
                # flow, but the reciprocal is a cheap cross-partition copy +
                # reciprocal_approx_fast (~1.3us) instead of the 3.3us
                # iterative divide, and the PSUM stage copy runs on ScalarE
                for h, ctx in ((hA, ctxA), (hB, ctxB)):
                    stage = stp.tile([HD + 1, 512], f32, tag="stage")
                    nc.scalar.copy(stage[:], ctx[:])
                    den = fp.tile([1, 512], f32, tag="den")
                    nc.vector.tensor_copy(den[:], stage[HD : HD + 1, :])
                    recip = fp.tile([1, 512], f32, tag="recip")
                    nc.vector.reciprocal_approx_fast(recip[:], den[:])
                    bcast = fp.tile([64, 512], f32, tag="bcast")
                    nc.gpsimd.partition_broadcast(bcast[:], recip[:])
                    out_sb = fp.tile([64, 512], f32, tag="out")
                    nc.vector.tensor_mul(out_sb[:], stage[0:HD, :], bcast[:])
                    rp = (h % 2) * 64
                    nc.vector.tensor_scalar_add(
                        out_sb[:], out_sb[:], bv_sb[rp : rp + 64, h // 2 : h // 2 + 1]
                    )
                    nc.sync.dma_start(out_d[ds(h * 64, 64), ds(qq * 512, 512)], out_sb[:])

            # ---- pass A: K, V projections (f-tile 0) + Q chunk 0 only;
            # each pair stages its own remaining Q chunks one quarter ahead,
            # which shortens pass A and gives pair 3 PE filler work ----
            for n in range(4):
                qk_proj(wkt, k_sb, 0, n)
                for m in range(4):
                    v_proj(m, n)
            qk_proj(wqt, q_sb, 0, 0, with_bias=True)
            wq_own = wqt

            # ---- attention pair p overlapped with projections for p+1:
            # per quarter, one own-pair Q group (needed next quarter) plus
            # one next-pair group (K first - attention qq=0 of the next pair
            # reads ALL K n-chunks; its Q chunk 0 last). Weights prefetched
            # a full pair ahead so proj matmuls at the head of the PE queue
            # never wait on the (congested) sync DMA queue. ----
            wkt_n = load_w_tile(wk_d, 1)
            wqt_n = load_w_tile(wq_d, 1)
            for p in range(4):
                wkt_cur, wqt_cur = wkt_n, wqt_n
                if p < 2:
                    wkt_n = load_w_tile(wk_d, p + 2)
                    wqt_n = load_w_tile(wq_d, p + 2)
                for qq in range(4):
                    if qq < 3:
                        # own Q chunk qq+1, consumed by the next quarter
                        qk_proj(wq_own, q_sb, p, qq + 1, with_bias=True)
                    if p < 3:
                        if qq < 3:
                            qk_proj(wkt_cur, k_sb, p + 1, qq)
                        else:
                            qk_proj(wkt_cur, k_sb, p + 1, 3)
                            qk_proj(wqt_cur, q_sb, p + 1, 0, with_bias=True)
                    attn_pair(p, qq)
                wq_own = wqt_cur

    nc.compile()
    return nc


def make_in_maps(
    hidden_states, attention_mask, q_weight, q_bias, k_weight, k_bias, v_weight, v_bias
):
    hs = np.asarray(hidden_states, dtype=np.float32)
    am = np.asarray(attention_mask, dtype=np.float32)
    ws = {
        "q": np.asarray(q_weight, dtype=np.float32),
        "k": np.asarray(k_weight, dtype=np.float32),
        "v": np.asarray(v_weight, dtype=np.float32),
    }
    bs = {
        "q": np.asarray(q_bias, dtype=np.float32),
        "k": np.asarray(k_bias, dtype=np.float32),
        "v": np.asarray(v_bias, dtype=np.float32),
    }
    in_maps = []
    for core in range(NC):
        b, half = divmod(core, 2)
        fsl = slice(half * F, (half + 1) * F)
        in_maps.append(
            {
                "x_t": np.ascontiguousarray(
                    hs[b].T.reshape(DM, 4, 512).transpose(1, 0, 2).reshape(4 * DM, 512)
                ).astype(_bf16np),
                "wq_t": np.ascontiguousarray(
                    ws["q"][fsl, :].T.reshape(DM, 4, 128).transpose(1, 0, 2).reshape(4 * DM, 128)
                ).astype(_bf16np),
                "wk_t": np.ascontiguousarray(
                    ws["k"][fsl, :].T.reshape(DM, 4, 128).transpose(1, 0, 2).reshape(4 * DM, 128)
                ).astype(_bf16np),
                "wv_t": np.ascontiguousarray(ws["v"][fsl, :].T).astype(_bf16np),
                "bqr": np.ascontiguousarray(bs["q"][fsl]).reshape(1, F).astype(_bf16np),
                "bv": np.ascontiguousarray(bs["v"][fsl]).reshape(F, 1),
                "mask": np.ascontiguousarray(am[b, 0, 0, :].reshape(16, 128).T),
            }
        )
    return in_maps


def assemble_out(results):
    out = np.empty((4, S, DM), dtype=np.float32)
    for core in range(NC):
        b, half = divmod(core, 2)
        out[b, :, half * F : (half + 1) * F] = results[core]["out_t"].T
    return out


_NC_CACHE = []


def _run(inputs, trace=False):
    from concourse.bass_utils import run_bass_kernel_spmd

    if not _NC_CACHE:
        _NC_CACHE.append(build_nc())
    nc = _NC_CACHE[0]
    in_maps = make_in_maps(**inputs)
    res = run_bass_kernel_spmd(nc, in_maps, list(range(NC)), trace=trace)
    return assemble_out(res.results), res


def kernel(**inputs):
    out, _ = _run(inputs, trace=False)
    return out



# revision 79
# speedup vs baseline: 1.0016x; 1.0016x over previous
"""BertSelfAttention (B=4, S=2048, D=1024, H=16, hd=64) on 8 trn2 NeuronCores.

Sharding: core = 2*b + half. Each core handles batch b = core//2 and 8 of the
16 heads (feature slice half*512 .. half*512+512). Fully embarrassingly
parallel: no collectives.

Per-core kernel (bf16 operands, fp32 PSUM accumulation; measured 366.5us HW
exec, absmax rel err 9.6e-3 vs the fp32 reference, gate 2e-2):
  X^T is resident in SBUF (32KB/partition), loaded once across three DMA
  queues (first chunk per-c so the first matmul waits on 64KB only).
  Pass A: K (f-tile 0) + V projections + Q chunk 0; each pair stages its
    own remaining Q chunks one quarter ahead during attention, which
    shortens pass A and gives every quarter (incl. pair 3's) PE filler.
    Q^T, K^T in [f, s] layout (head dim on partitions); V in [s, f] layout
    with a ones column per head so the PV matmul also accumulates the
    softmax denominator in PSUM row 64. The Q bias is applied as a rank-1
    PE accumulation (bq x ones) into the projection PSUM and the K bias is
    dropped (it shifts scores by a per-query constant, which softmax
    cancels exactly), so projection staging is a plain ScalarE copy and
    attention never depends on the congested DVE queue.
  Attention per head-pair p, per q-quarter qq (512 wide):
    S^T chunks for both heads land in one [128, 1024] PSUM tile (head A in
    cols 0:512 via PE row-group 0-63, head B in cols 512:1024 via row-group
    64-127). Emission is software-pipelined in chunk pairs - the PE runs
    [PV(2t-2) PV(2t-1) QK(2t+2) QK(2t+3)] while both exp engines process
    chunks 2t/2t+1 concurrently, breaking the serial
    exp(c)->PV(c)->QK(c+1)->exp(c+1) chain. The exp work is split so
    neither engine is the wall (ScalarE alone would be 293us):
      - ScalarE chunks (10/16): exact exp activation (mask as per-partition
        bias, 1/sqrt(64) scale folded), bf16 out.
      - DVE chunks (6/16, odd slots so pairs are (ScalarE, DVE)):
        Schraudolph exp - one tensor_scalar computing
        round(23.08312*s + (184.665*mask + 16250)) into int16, which IS the
        bf16 bit pattern of ~exp(s/8 + mask). The constant-bias ratio error
        cancels exactly in the softmax ratio; the +-3% sawtooth residual
        keeps final rel err ~1e-2.
    PV accumulates ctx^T (rows 0..63) + denominator (row 64) over the 16
    k-chunks. Finalize per head: stage ctx out of PSUM on ScalarE (frees
    the PSUM bank fast, keeps DVE on exp), cross-partition copy of the den
    row + reciprocal_approx_fast (~5x cheaper than the iterative-divide
    reciprocal; the custom op needs a partition-0 input on HW), gpsimd
    partition-broadcast, DVE multiply + bias add, DMA out.
  Projection weights are prefetched a full pair ahead so the proj matmuls
  at the head of the PE queue never wait on the sync DMA queue; ~26 dummy
  matmuls during the initial DMA wait keep the HAM activity window busy so
  the first real matmuls run at 2.4GHz.
"""

import numpy as np
from ml_dtypes import bfloat16 as _bf16np

S = 2048  # sequence length
DM = 1024  # model dim
F = 512  # features per core (8 heads x 64)
HL = 8  # heads per core
HD = 64  # head dim
NC = 8  # cores

# k-chunks (of 16 per q-quarter) whose exp runs on DVE via the Schraudolph
# bit trick instead of ScalarE's exact exp. Placed at the odd slot of a
# chunk-pair so most pairs are (ScalarE, DVE) and the two engines overlap.
# Quarters where the DVE also carries finalize work get fewer trick chunks
# so the in-order DVE queue never stalls the PE's PV matmuls.
def dve_chunks(p, qq):
    return frozenset((1, 3, 5, 9, 11, 13))
# bf16 bit pattern of exp(y) ~= 184.66496*y + 16250 (128/ln2; bias tuned so
# values center in range - any constant offset cancels in the softmax ratio)
EXP_MUL = 184.66496 * 0.125  # scores arrive unscaled; fold the 1/sqrt(64)
EXP_BIAS = 16250.0


def build_nc():
    import concourse.bass as bass
    import concourse.mybir as mybir
    import concourse.tile as tile
    from concourse import bacc
    from concourse.bass import ds, ts

    f32 = mybir.dt.float32
    f32r = mybir.dt.float32r
    bf16 = mybir.dt.bfloat16
    i16 = mybir.dt.int16
    EXP = mybir.ActivationFunctionType.Exp
    PSUM = bass.MemorySpace.PSUM

    nc = bacc.Bacc("TRN2", target_bir_lowering=False, debug=False, num_devices=NC)

    x_d = nc.declare_dram_parameter("x_t", [4 * DM, 512], bf16, isOutput=False)
    bqr_d = nc.declare_dram_parameter("bqr", [1, F], bf16, isOutput=False)
    wq_d = nc.declare_dram_parameter("wq_t", [4 * DM, 128], bf16, isOutput=False)
    wk_d = nc.declare_dram_parameter("wk_t", [4 * DM, 128], bf16, isOutput=False)
    wv_d = nc.declare_dram_parameter("wv_t", [DM, F], bf16, isOutput=False)
    bv_d = nc.declare_dram_parameter("bv", [F, 1], f32, isOutput=False)
    mask_d = nc.declare_dram_parameter("mask", [128, 16], f32, isOutput=False)
    out_d = nc.declare_dram_parameter("out_t", [F, S], f32, isOutput=True)

    mm = nc.tensor.matmul

    with tile.TileContext(nc) as tc:
        with (
            tc.tile_pool(name="const", bufs=1) as const,
            tc.tile_pool(name="w", bufs=1) as wpool,
            tc.tile_pool(name="wqk", bufs=4) as wqkp,
            tc.tile_pool(name="qkv", bufs=1) as qkv,
            tc.tile_pool(name="pqkv", bufs=2, space=PSUM) as pqkv,
            tc.tile_pool(name="s_ps", bufs=2, space=PSUM) as sp,
            tc.tile_pool(name="ctxA", bufs=1, space=PSUM) as cpA,
            tc.tile_pool(name="ctxB", bufs=1, space=PSUM) as cpB,
            tc.tile_pool(name="expp", bufs=12) as ep,
            tc.tile_pool(name="stagep", bufs=14) as stp,
            tc.tile_pool(name="fin", bufs=4) as fp,
        ):
            # critical-path first: wk/wq f-tile 0 and x chunk 0 on the sync
            # queue before anything else touches DMA
            def load_w_tile(w_d, i):
                # per-c DMAs so the first proj matmul only waits for 64KB
                wt = wqkp.tile([128, 8, 128], bf16, tag="wt")
                for c in range(8):
                    nc.sync.dma_start(wt[:, c, :], w_d[ds(i * DM + c * 128, 128), :])
                return wt

            # X^T resident in SBUF for the whole kernel (32KB/partition):
            # x_sb[p, c, n, s'] = X^T[c*128+p, n*512+s']
            x_sb = qkv.tile([128, 8, 4, 512], bf16)

            def x_load(n, eng):
                # one strided DMA per chunk (descriptor issue costs ~0.7us -
                # per-c splitting jams the issuing queue), spread over
                # different engines' DMA queues so the ~15us transfers of
                # the four chunks run in parallel
                eng.dma_start(
                    x_sb[:, :, n, :],
                    x_d[ds(n * DM, DM), :].rearrange("(c p) s -> p c s", p=128),
                )

            # first chunk per-c so the very first projection matmul only
            # waits on one 64KB transfer, not the whole 512KB chunk
            for c in range(8):
                nc.gpsimd.dma_start(x_sb[:, c, 0, :], x_d[ds(c * 128, 128), :])
            # chunk 1 split across two queues - K(0,1) needs it ~6us after
            # pass A starts, a full-chunk transfer on one queue arrives late
            nc.scalar.dma_start(
                x_sb[:, 0:4, 1, :],
                x_d[ds(DM, 512), :].rearrange("(c p) s -> p c s", p=128),
            )
            nc.gpsimd.dma_start(
                x_sb[:, 4:8, 1, :],
                x_d[ds(DM + 512, 512), :].rearrange("(c p) s -> p c s", p=128),
            )
            x_load(2, nc.gpsimd)
            wkt = load_w_tile(wk_d, 0)
            wqt = load_w_tile(wq_d, 0)
            x_load(3, nc.sync)  # after the weight tiles on the sync queue

            # memset can't emit float32r directly; memset f32 then round-copy
            ones_f32 = const.tile([128, 128], f32)
            nc.vector.memset(ones_f32[:], 1.0)
            warm = const.tile([1, 1], f32)
            nc.scalar.activation(warm[:], ones_f32[0:1, 0:1], EXP)
            # warm the PE during the initial DMA wait: ~26 dummy matmuls keep
            # the HAM activity window busy so the first real matmuls run at
            # 2.4GHz instead of paying the 1.2GHz cold penalty
            warm_bf = const.tile([128, 512], bf16)
            nc.vector.memset(warm_bf[:], 1.0)
            warm_ps = pqkv.tile([128, 512], f32, tag="pqkv")
            for i in range(26):
                mm(
                    warm_ps[:],
                    warm_bf[:, 0:128],
                    warm_bf[:],
                    start=(i == 0),
                    stop=(i == 25),
                )
            wv_sb = wpool.tile([128, 8, F], bf16)
            for c in range(8):
                nc.gpsimd.dma_start(wv_sb[:, c, :], wv_d[ts(c, 128), :])

            # Q bias as a [1, 512] row (host-prepared) for the rank-1 PE
            # accumulation into the projection PSUM; K bias is dropped - it
            # only shifts scores by a per-query constant, which softmax
            # cancels exactly. A ones row is the rank-1 matmul's rhs.
            bqr_sb = const.tile([1, F], bf16)
            nc.gpsimd.dma_start(bqr_sb[:], bqr_d[:])
            ones_row = const.tile([1, 512], bf16)
            nc.vector.memset(ones_row[:], 1.0)
            bv_sb = const.tile([128, 4], f32)
            for i in range(4):
                nc.gpsimd.dma_start(bv_sb[:, i : i + 1], bv_d[ts(i, 128), :])
            mask_sb = const.tile([128, 16], f32)
            nc.gpsimd.dma_start(mask_sb[:], mask_d[:])
            # per-(k-partition, chunk) additive constant for the DVE exp trick
            c16_sb = const.tile([128, 16], f32)
            nc.vector.tensor_scalar(
                out=c16_sb[:],
                in0=mask_sb[:],
                scalar1=184.66496,
                scalar2=EXP_BIAS,
                op0=mybir.AluOpType.mult,
                op1=mybir.AluOpType.add,
            )

            # Q^T / K^T: [f, s] layout as 4 partition tiles of 128 features.
            q_sb = qkv.tile([128, 4, S], bf16)
            k_sb = qkv.tile([128, 4, S], bf16)
            # V in [k, head, d+1] layout; column 64 = 1.0 (denominator trick).
            v_sb = qkv.tile([128, 16, HL, HD + 1], bf16)
            nc.vector.tensor_copy(
                v_sb[:, :, :, HD], ones_f32[:, 0:128].rearrange("p (a b) -> p a b", a=16)
            )

            def qk_proj(wt, dst, i, n, with_bias=False):
                ps = pqkv.tile([128, 512], f32, tag="pqkv")
                for c in range(8):
                    mm(
                        ps[:],
                        wt[:, c, :],
                        x_sb[:, c, n, :],
                        start=(c == 0),
                        stop=(c == 7) and not with_bias,
                    )
                if with_bias:
                    # bias as rank-1 accumulation: ps += bq[f-tile] x ones
                    mm(
                        ps[:],
                        bqr_sb[0:1, ts(i, 128)],
                        ones_row[:],
                        start=False,
                        stop=True,
                    )
                # plain copy -> ScalarE, so the next pair's QK matmuls never
                # depend on the congested DVE queue
                nc.scalar.copy(dst[:, i, ts(n, 512)], ps[:])

            def v_proj(m, n, on_scalar=False):
                kc = n * 4 + m
                ps = pqkv.tile([128, 512], f32, tag="pqkv")
                for c in range(8):
                    mm(
                        ps[:],
                        x_sb[:, c, n, ts(m, 128)],
                        wv_sb[:, c, :],
                        start=(c == 0),
                        stop=(c == 7),
                    )
                dst = v_sb[:, kc, :, 0:HD]
                src = ps[:].rearrange("p (h d) -> p h d", h=HL)
                if on_scalar:
                    # keeps the DVE queue free for the exp trick when V is
                    # staged just-in-time inside attn(0,0)
                    nc.scalar.copy(dst, src)
                else:
                    nc.vector.tensor_copy(dst, src)

            def attn_pair(p, qq):
                hA, hB = 2 * p, 2 * p + 1
                qsl = ds(qq * 512, 512)
                ctxA = cpA.tile([HD + 1, 512], f32, tag="cA")
                ctxB = cpB.tile([HD + 1, 512], f32, tag="cB")
                def emit_qk(c):
                    sps = sp.tile([128, 1024], f32, tag="s")
                    mm(
                        sps[:, 0:512],
                        k_sb[0:64, p, ds(c * 128, 128)],
                        q_sb[0:64, p, qsl],
                        start=True,
                        stop=True,
                        tile_position=(0, 0),
                    )
                    mm(
                        sps[:, 512:1024],
                        k_sb[64:128, p, ds(c * 128, 128)],
                        q_sb[64:128, p, qsl],
                        start=True,
                        stop=True,
                        tile_position=(64, 0),
                    )
                    return sps

                dchunks = dve_chunks(p, qq)

                def emit_exp(c, sps):
                    et = ep.tile([128, 1024], bf16, tag="e")
                    if c in dchunks:
                        nc.vector.tensor_scalar(
                            out=et[:].bitcast(i16),
                            in0=sps[:],
                            scalar1=EXP_MUL,
                            scalar2=c16_sb[:, c : c + 1],
                            op0=mybir.AluOpType.mult,
                            op1=mybir.AluOpType.add,
                        )
                    else:
                        nc.scalar.activation(
                            et[:], sps[:], EXP, bias=mask_sb[:, c : c + 1], scale=0.125
                        )
                    return et

                def emit_pv(c, et):
                    mm(
                        ctxA[:],
                        v_sb[:, c, hA, :],
                        et[:, 0:512],
                        start=(c == 0),
                        stop=(c == 15),
                    )
                    mm(
                        ctxB[:],
                        v_sb[:, c, hB, :],
                        et[:, 512:1024],
                        start=(c == 0),
                        stop=(c == 15),
                    )

                # software-pipelined emission in chunk pairs: the PE queue
                # runs [PV(2t-2) PV(2t-1) QK(2t+2) QK(2t+3)] while both exp
                # engines process chunks 2t/2t+1 concurrently - the serial
                # chain exp(c)->PV(c)->QK(c+1)->exp(c+1) is broken.
                ets = {}
                for t in range(8):
                    s0 = emit_qk(2 * t)
                    s1 = emit_qk(2 * t + 1)
                    ets[2 * t] = emit_exp(2 * t, s0)
                    ets[2 * t + 1] = emit_exp(2 * t + 1, s1)
                    if t > 0:
                        emit_pv(2 * t - 2, ets.pop(2 * t - 2))
                        emit_pv(2 * t - 1, ets.pop(2 * t - 1))
                emit_pv(14, ets.pop(14))
                emit_pv(15, ets.pop(15))
                # finalize inline per head, exactly like the proven baseline
                # flow, but the reciprocal is a cheap cross-partition copy +
                # reciprocal_approx_fast (~1.3us) instead of the 3.3us
                # iterative divide, and the PSUM stage copy runs on ScalarE
                for h, ctx in ((hA, ctxA), (hB, ctxB)):
                    stage = stp.tile([HD + 1, 512], f32, tag="stage")
                    nc.scalar.copy(stage[:], ctx[:])
                    den = fp.tile([1, 512], f32, tag="den")
                    nc.vector.tensor_copy(den[:], stage[HD : HD + 1, :])
                    recip = fp.tile([1, 512], f32, tag="recip")
                    nc.vector.reciprocal_approx_fast(recip[:], den[:])
                    bcast = fp.tile([64, 512], f32, tag="bcast")
                    nc.gpsimd.partition_broadcast(bcast[:], recip[:])
                    out_sb = fp.tile([64, 512], f32, tag="out")
                    nc.vector.tensor_mul(out_sb[:], stage[0:HD, :], bcast[:])
                    rp = (h % 2) * 64
                    nc.vector.tensor_scalar_add(
                        out_sb[:], out_sb[:], bv_sb[rp : rp + 64, h // 2 : h // 2 + 1]
                    )
                    nc.sync.dma_start(out_d[ds(h * 64, 64), ds(qq * 512, 512)], out_sb[:])

            # ---- pass A: K, V projections (f-tile 0) + Q chunk 0 only;
            # each pair stages its own remaining Q chunks one quarter ahead,
            # which shortens pass A and gives pair 3 PE filler work ----
            for n in range(4):
                qk_proj(wkt, k_sb, 0, n)
                for m in range(4):
                    v_proj(m, n)
            qk_proj(wqt, q_sb, 0, 0, with_bias=True)
            wq_own = wqt

            # ---- attention pair p overlapped with projections for p+1:
            # per quarter, one own-pair Q group (needed next quarter) plus
            # one next-pair group (K first - attention qq=0 of the next pair
            # reads ALL K n-chunks; its Q chunk 0 last). Weights prefetched
            # a full pair ahead so proj matmuls at the head of the PE queue
            # never wait on the (congested) sync DMA queue. ----
            wkt_n = load_w_tile(wk_d, 1)
            wqt_n = load_w_tile(wq_d, 1)
            for p in range(4):
                wkt_cur, wqt_cur = wkt_n, wqt_n
                if p < 2:
                    wkt_n = load_w_tile(wk_d, p + 2)
                    wqt_n = load_w_tile(wq_d, p + 2)
                for qq in range(4):
                    if qq < 3:
                        # own Q chunk qq+1, consumed by the next quarter
                        qk_proj(wq_own, q_sb, p, qq + 1, with_bias=True)
                    if p < 3:
                        if qq < 3:
                            qk_proj(wkt_cur, k_sb, p + 1, qq)
                        else:
                            qk_proj(wkt_cur, k_sb, p + 1, 3)
                            qk_proj(wqt_cur, q_sb, p + 1, 0, with_bias=True)
                    attn_pair(p, qq)
                wq_own = wqt_cur

    nc.compile()
    return nc


def make_in_maps(
    hidden_states, attention_mask, q_weight, q_bias, k_weight, k_bias, v_weight, v_bias
):
    hs = np.asarray(hidden_states, dtype=np.float32)
    am = np.asarray(attention_mask, dtype=np.float32)
    ws = {
        "q": np.asarray(q_weight, dtype=np.float32),
        "k": np.asarray(k_weight, dtype=np.float32),
        "v": np.asarray(v_weight, dtype=np.float32),
    }
    bs = {
        "q": np.asarray(q_bias, dtype=np.float32),
        "k": np.asarray(k_bias, dtype=np.float32),
        "v": np.asarray(v_bias, dtype=np.float32),
    }
    in_maps = []
    for core in range(NC):
        b, half = divmod(core, 2)
        fsl = slice(half * F, (half + 1) * F)
        in_maps.append(
            {
                "x_t": np.ascontiguousarray(
                    hs[b].T.reshape(DM, 4, 512).transpose(1, 0, 2).reshape(4 * DM, 512)
                ).astype(_bf16np),
                "wq_t": np.ascontiguousarray(
                    ws["q"][fsl, :].T.reshape(DM, 4, 128).transpose(1, 0, 2).reshape(4 * DM, 128)
                ).astype(_bf16np),
                "wk_t": np.ascontiguousarray(
                    ws["k"][fsl, :].T.reshape(DM, 4, 128).transpose(1, 0, 2).reshape(4 * DM, 128)
                ).astype(_bf16np),
                "wv_t": np.ascontiguousarray(ws["v"][fsl, :].T).astype(_bf16np),
                "bqr": np.ascontiguousarray(bs["q"][fsl]).reshape(1, F).astype(_bf16np),
                "bv": np.ascontiguousarray(bs["v"][fsl]).reshape(F, 1),
                "mask": np.ascontiguousarray(am[b, 0, 0, :].reshape(16, 128).T),
            }
        )
    return in_maps


def assemble_out(results):
    out = np.empty((4, S, DM), dtype=np.float32)
    for core in range(NC):
        b, half = divmod(core, 2)
        out[b, :, half * F : (half + 1) * F] = results[core]["out_t"].T
    return out


_NC_CACHE = []


def _run(inputs, trace=False):
    from concourse.bass_utils import run_bass_kernel_spmd

    if not _NC_CACHE:
        _NC_CACHE.append(build_nc())
    nc = _NC_CACHE[0]
    in_maps = make_in_maps(**inputs)
    res = run_bass_kernel_spmd(nc, in_maps, list(range(NC)), trace=trace)
    return assemble_out(res.results), res


def kernel(**inputs):
    out, _ = _run(inputs, trace=False)
    return out



# revision 81
# speedup vs baseline: 1.0126x; 1.0109x over previous
"""BertSelfAttention (B=4, S=2048, D=1024, H=16, hd=64) on 8 trn2 NeuronCores.

Sharding: core = 2*b + half. Each core handles batch b = core//2 and 8 of the
16 heads (feature slice half*512 .. half*512+512). Fully embarrassingly
parallel: no collectives.

Per-core kernel (bf16 operands, fp32 PSUM accumulation; measured 366.5us HW
exec, absmax rel err 9.6e-3 vs the fp32 reference, gate 2e-2):
  X^T is resident in SBUF (32KB/partition), loaded once across three DMA
  queues (first chunk per-c so the first matmul waits on 64KB only).
  Pass A: K (f-tile 0) + V projections + Q chunk 0; each pair stages its
    own remaining Q chunks one quarter ahead during attention, which
    shortens pass A and gives every quarter (incl. pair 3's) PE filler.
    Q^T, K^T in [f, s] layout (head dim on partitions); V in [s, f] layout
    with a ones column per head so the PV matmul also accumulates the
    softmax denominator in PSUM row 64. The Q bias is applied as a rank-1
    PE accumulation (bq x ones) into the projection PSUM and the K bias is
    dropped (it shifts scores by a per-query constant, which softmax
    cancels exactly), so projection staging is a plain ScalarE copy and
    attention never depends on the congested DVE queue.
  Attention per head-pair p, per q-quarter qq (512 wide):
    S^T chunks for both heads land in one [128, 1024] PSUM tile (head A in
    cols 0:512 via PE row-group 0-63, head B in cols 512:1024 via row-group
    64-127). Emission is software-pipelined in chunk pairs - the PE runs
    [PV(2t-2) PV(2t-1) QK(2t+2) QK(2t+3)] while both exp engines process
    chunks 2t/2t+1 concurrently, breaking the serial
    exp(c)->PV(c)->QK(c+1)->exp(c+1) chain. The exp work is split so
    neither engine is the wall (ScalarE alone would be 293us):
      - ScalarE chunks (10/16): exact exp activation (mask as per-partition
        bias, 1/sqrt(64) scale folded), bf16 out.
      - DVE chunks (6/16, odd slots so pairs are (ScalarE, DVE)):
        Schraudolph exp - one tensor_scalar computing
        round(23.08312*s + (184.665*mask + 16250)) into int16, which IS the
        bf16 bit pattern of ~exp(s/8 + mask). The constant-bias ratio error
        cancels exactly in the softmax ratio; the +-3% sawtooth residual
        keeps final rel err ~1e-2.
    PV accumulates ctx^T (rows 0..63) + denominator (row 64) over the 16
    k-chunks. Finalize per head: stage ctx out of PSUM on ScalarE (frees
    the PSUM bank fast, keeps DVE on exp), cross-partition copy of the den
    row + reciprocal_approx_fast (~5x cheaper than the iterative-divide
    reciprocal; the custom op needs a partition-0 input on HW), gpsimd
    partition-broadcast, DVE multiply + bias add, DMA out.
  Projection weights are prefetched a full pair ahead so the proj matmuls
  at the head of the PE queue never wait on the sync DMA queue; ~26 dummy
  matmuls during the initial DMA wait keep the HAM activity window busy so
  the first real matmuls run at 2.4GHz.
"""

import numpy as np
from ml_dtypes import bfloat16 as _bf16np

S = 2048  # sequence length
DM = 1024  # model dim
F = 512  # features per core (8 heads x 64)
HL = 8  # heads per core
HD = 64  # head dim
NC = 8  # cores

# k-chunks (of 16 per q-quarter) whose exp runs on DVE via the Schraudolph
# bit trick instead of ScalarE's exact exp. Placed at the odd slot of a
# chunk-pair so most pairs are (ScalarE, DVE) and the two engines overlap.
# Quarters where the DVE also carries finalize work get fewer trick chunks
# so the in-order DVE queue never stalls the PE's PV matmuls.
def dve_chunks(p, qq):
    return frozenset((1, 3, 5, 9, 11, 13))
# bf16 bit pattern of exp(y) ~= 184.66496*y + 16250 (128/ln2; bias tuned so
# values center in range - any constant offset cancels in the softmax ratio)
EXP_MUL = 184.66496 * 0.125  # scores arrive unscaled; fold the 1/sqrt(64)
EXP_BIAS = 16250.0


def build_nc():
    import concourse.bass as bass
    import concourse.mybir as mybir
    import concourse.tile as tile
    from concourse import bacc
    from concourse.bass import ds, ts

    f32 = mybir.dt.float32
    f32r = mybir.dt.float32r
    bf16 = mybir.dt.bfloat16
    i16 = mybir.dt.int16
    EXP = mybir.ActivationFunctionType.Exp
    PSUM = bass.MemorySpace.PSUM

    nc = bacc.Bacc("TRN2", target_bir_lowering=False, debug=False, num_devices=NC)

    x_d = nc.declare_dram_parameter("x_t", [4 * DM, 512], bf16, isOutput=False)
    bqr_d = nc.declare_dram_parameter("bqr", [1, F], bf16, isOutput=False)
    wq_d = nc.declare_dram_parameter("wq_t", [4 * DM, 128], bf16, isOutput=False)
    wk_d = nc.declare_dram_parameter("wk_t", [4 * DM, 128], bf16, isOutput=False)
    wv_d = nc.declare_dram_parameter("wv_t", [DM, F], bf16, isOutput=False)
    bv_d = nc.declare_dram_parameter("bv", [F, 1], f32, isOutput=False)
    mask_d = nc.declare_dram_parameter("mask", [128, 16], f32, isOutput=False)
    out_d = nc.declare_dram_parameter("out_t", [F, S], f32, isOutput=True)

    mm = nc.tensor.matmul

    with tile.TileContext(nc) as tc:
        with (
            tc.tile_pool(name="const", bufs=1) as const,
            tc.tile_pool(name="w", bufs=1) as wpool,
            tc.tile_pool(name="wqk", bufs=4) as wqkp,
            tc.tile_pool(name="qkv", bufs=1) as qkv,
            tc.tile_pool(name="pqkv", bufs=2, space=PSUM) as pqkv,
            tc.tile_pool(name="s_ps", bufs=2, space=PSUM) as sp,
            tc.tile_pool(name="ctxA", bufs=1, space=PSUM) as cpA,
            tc.tile_pool(name="ctxB", bufs=1, space=PSUM) as cpB,
            tc.tile_pool(name="expp", bufs=12) as ep,
            tc.tile_pool(name="stagep", bufs=14) as stp,
            tc.tile_pool(name="fin", bufs=4) as fp,
        ):
            # critical-path first: wk/wq f-tile 0 and x chunk 0 on the sync
            # queue before anything else touches DMA
            def load_w_tile(w_d, i):
                # per-c DMAs so the first proj matmul only waits for 64KB
                wt = wqkp.tile([128, 8, 128], bf16, tag="wt")
                for c in range(8):
                    nc.sync.dma_start(wt[:, c, :], w_d[ds(i * DM + c * 128, 128), :])
                return wt

            # X^T resident in SBUF for the whole kernel (32KB/partition):
            # x_sb[p, c, n, s'] = X^T[c*128+p, n*512+s']
            x_sb = qkv.tile([128, 8, 4, 512], bf16)

            def x_load(n, eng):
                # one strided DMA per chunk (descriptor issue costs ~0.7us -
                # per-c splitting jams the issuing queue), spread over
                # different engines' DMA queues so the ~15us transfers of
                # the four chunks run in parallel
                eng.dma_start(
                    x_sb[:, :, n, :],
                    x_d[ds(n * DM, DM), :].rearrange("(c p) s -> p c s", p=128),
                )

            # first chunk per-c so the very first projection matmul only
            # waits on one 64KB transfer, not the whole 512KB chunk
            for c in range(8):
                nc.gpsimd.dma_start(x_sb[:, c, 0, :], x_d[ds(c * 128, 128), :])
            # V weights next on gpsimd - the first V projection group needs
            # all 8 wv chunks ~4us after pass A starts
            wv_sb = wpool.tile([128, 8, F], bf16)
            for c in range(8):
                nc.gpsimd.dma_start(wv_sb[:, c, :], wv_d[ts(c, 128), :])
            # chunk 1 split across two queues - K(0,1) needs it ~9us after
            # pass A starts, a full-chunk transfer on one queue arrives late
            nc.scalar.dma_start(
                x_sb[:, 0:4, 1, :],
                x_d[ds(DM, 512), :].rearrange("(c p) s -> p c s", p=128),
            )
            nc.gpsimd.dma_start(
                x_sb[:, 4:8, 1, :],
                x_d[ds(DM + 512, 512), :].rearrange("(c p) s -> p c s", p=128),
            )
            x_load(2, nc.gpsimd)
            wkt = load_w_tile(wk_d, 0)
            wqt = load_w_tile(wq_d, 0)
            x_load(3, nc.sync)  # after the weight tiles on the sync queue

            # memset can't emit float32r directly; memset f32 then round-copy
            ones_f32 = const.tile([128, 128], f32)
            nc.vector.memset(ones_f32[:], 1.0)
            warm = const.tile([1, 1], f32)
            nc.scalar.activation(warm[:], ones_f32[0:1, 0:1], EXP)
            # warm the PE during the initial DMA wait: ~26 dummy matmuls keep
            # the HAM activity window busy so the first real matmuls run at
            # 2.4GHz instead of paying the 1.2GHz cold penalty
            warm_bf = const.tile([128, 512], bf16)
            nc.vector.memset(warm_bf[:], 1.0)
            warm_ps = pqkv.tile([128, 512], f32, tag="pqkv")
            for i in range(26):
                mm(
                    warm_ps[:],
                    warm_bf[:, 0:128],
                    warm_bf[:],
                    start=(i == 0),
                    stop=(i == 25),
                )

            # Q bias as a [1, 512] row (host-prepared) for the rank-1 PE
            # accumulation into the projection PSUM; K bias is dropped - it
            # only shifts scores by a per-query constant, which softmax
            # cancels exactly. A ones row is the rank-1 matmul's rhs.
            bqr_sb = const.tile([1, F], bf16)
            nc.gpsimd.dma_start(bqr_sb[:], bqr_d[:])
            ones_row = const.tile([1, 512], bf16)
            nc.vector.memset(ones_row[:], 1.0)
            bv_sb = const.tile([128, 4], f32)
            for i in range(4):
                nc.gpsimd.dma_start(bv_sb[:, i : i + 1], bv_d[ts(i, 128), :])
            mask_sb = const.tile([128, 16], f32)
            nc.gpsimd.dma_start(mask_sb[:], mask_d[:])
            # per-(k-partition, chunk) additive constant for the DVE exp trick
            c16_sb = const.tile([128, 16], f32)
            nc.vector.tensor_scalar(
                out=c16_sb[:],
                in0=mask_sb[:],
                scalar1=184.66496,
                scalar2=EXP_BIAS,
                op0=mybir.AluOpType.mult,
                op1=mybir.AluOpType.add,
            )

            # Q^T / K^T: [f, s] layout as 4 partition tiles of 128 features.
            q_sb = qkv.tile([128, 4, S], bf16)
            k_sb = qkv.tile([128, 4, S], bf16)
            # V in [k, head, d+1] layout; column 64 = 1.0 (denominator trick).
            v_sb = qkv.tile([128, 16, HL, HD + 1], bf16)
            nc.vector.tensor_copy(
                v_sb[:, :, :, HD], ones_f32[:, 0:128].rearrange("p (a b) -> p a b", a=16)
            )

            def qk_proj(wt, dst, i, n, with_bias=False):
                ps = pqkv.tile([128, 512], f32, tag="pqkv")
                for c in range(8):
                    mm(
                        ps[:],
                        wt[:, c, :],
                        x_sb[:, c, n, :],
                        start=(c == 0),
                        stop=(c == 7) and not with_bias,
                    )
                if with_bias:
                    # bias as rank-1 accumulation: ps += bq[f-tile] x ones
                    mm(
                        ps[:],
                        bqr_sb[0:1, ts(i, 128)],
                        ones_row[:],
                        start=False,
                        stop=True,
                    )
                # plain copy -> ScalarE, so the next pair's QK matmuls never
                # depend on the congested DVE queue
                nc.scalar.copy(dst[:, i, ts(n, 512)], ps[:])

            def v_proj(m, n, on_scalar=False):
                kc = n * 4 + m
                ps = pqkv.tile([128, 512], f32, tag="pqkv")
                for c in range(8):
                    mm(
                        ps[:],
                        x_sb[:, c, n, ts(m, 128)],
                        wv_sb[:, c, :],
                        start=(c == 0),
                        stop=(c == 7),
                    )
                dst = v_sb[:, kc, :, 0:HD]
                src = ps[:].rearrange("p (h d) -> p h d", h=HL)
                if on_scalar:
                    # keeps the DVE queue free for the exp trick when V is
                    # staged just-in-time inside attn(0,0)
                    nc.scalar.copy(dst, src)
                else:
                    nc.vector.tensor_copy(dst, src)

            def attn_pair(p, qq):
                hA, hB = 2 * p, 2 * p + 1
                qsl = ds(qq * 512, 512)
                ctxA = cpA.tile([HD + 1, 512], f32, tag="cA")
                ctxB = cpB.tile([HD + 1, 512], f32, tag="cB")
                def emit_qk(c):
                    sps = sp.tile([128, 1024], f32, tag="s")
                    mm(
                        sps[:, 0:512],
                        k_sb[0:64, p, ds(c * 128, 128)],
                        q_sb[0:64, p, qsl],
                        start=True,
                        stop=True,
                        tile_position=(0, 0),
                    )
                    mm(
                        sps[:, 512:1024],
                        k_sb[64:128, p, ds(c * 128, 128)],
                        q_sb[64:128, p, qsl],
                        start=True,
                        stop=True,
                        tile_position=(64, 0),
                    )
                    return sps

                dchunks = dve_chunks(p, qq)

                def emit_exp(c, sps):
                    et = ep.tile([128, 1024], bf16, tag="e")
                    if c in dchunks:
                        nc.vector.tensor_scalar(
                            out=et[:].bitcast(i16),
                            in0=sps[:],
                            scalar1=EXP_MUL,
                            scalar2=c16_sb[:, c : c + 1],
                            op0=mybir.AluOpType.mult,
                            op1=mybir.AluOpType.add,
                        )
                    else:
                        nc.scalar.activation(
                            et[:], sps[:], EXP, bias=mask_sb[:, c : c + 1], scale=0.125
                        )
                    return et

                def emit_pv(c, et):
                    mm(
                        ctxA[:],
                        v_sb[:, c, hA, :],
                        et[:, 0:512],
                        start=(c == 0),
                        stop=(c == 15),
                    )
                    mm(
                        ctxB[:],
                        v_sb[:, c, hB, :],
                        et[:, 512:1024],
                        start=(c == 0),
                        stop=(c == 15),
                    )

                # software-pipelined emission in chunk pairs: the PE queue
                # runs [PV(2t-2) PV(2t-1) QK(2t+2) QK(2t+3)] while both exp
                # engines process chunks 2t/2t+1 concurrently - the serial
                # chain exp(c)->PV(c)->QK(c+1)->exp(c+1) is broken.
                ets = {}
                for t in range(8):
                    s0 = emit_qk(2 * t)
                    s1 = emit_qk(2 * t + 1)
                    ets[2 * t] = emit_exp(2 * t, s0)
                    ets[2 * t + 1] = emit_exp(2 * t + 1, s1)
                    if t > 0:
                        emit_pv(2 * t - 2, ets.pop(2 * t - 2))
                        emit_pv(2 * t - 1, ets.pop(2 * t - 1))
                emit_pv(14, ets.pop(14))
                emit_pv(15, ets.pop(15))
                # finalize inline per head, exactly like the proven baseline
                # flow, but the reciprocal is a cheap cross-partition copy +
                # reciprocal_approx_fast (~1.3us) instead of the 3.3us
                # iterative divide, and the PSUM stage copy runs on ScalarE
                for h, ctx in ((hA, ctxA), (hB, ctxB)):
                    stage = stp.tile([HD + 1, 512], f32, tag="stage")
                    nc.scalar.copy(stage[:], ctx[:])
                    den = fp.tile([1, 512], f32, tag="den")
                    nc.vector.tensor_copy(den[:], stage[HD : HD + 1, :])
                    recip = fp.tile([1, 512], f32, tag="recip")
                    nc.vector.reciprocal_approx_fast(recip[:], den[:])
                    bcast = fp.tile([64, 512], f32, tag="bcast")
                    nc.gpsimd.partition_broadcast(bcast[:], recip[:])
                    out_sb = fp.tile([64, 512], f32, tag="out")
                    nc.vector.tensor_mul(out_sb[:], stage[0:HD, :], bcast[:])
                    rp = (h % 2) * 64
                    nc.vector.tensor_scalar_add(
                        out_sb[:], out_sb[:], bv_sb[rp : rp + 64, h // 2 : h // 2 + 1]
                    )
                    nc.sync.dma_start(out_d[ds(h * 64, 64), ds(qq * 512, 512)], out_sb[:])

            # ---- pass A: K, V projections (f-tile 0) + Q chunk 0 only;
            # each pair stages its own remaining Q chunks one quarter ahead,
            # which shortens pass A and gives pair 3 PE filler work ----
            for n in range(4):
                qk_proj(wkt, k_sb, 0, n)
                for m in range(4):
                    v_proj(m, n)
            qk_proj(wqt, q_sb, 0, 0, with_bias=True)
            wq_own = wqt

            # ---- attention pair p overlapped with projections for p+1:
            # per quarter, one own-pair Q group (needed next quarter) plus
            # one next-pair group (K first - attention qq=0 of the next pair
            # reads ALL K n-chunks; its Q chunk 0 last). Weights prefetched
            # a full pair ahead so proj matmuls at the head of the PE queue
            # never wait on the (congested) sync DMA queue. ----
            wkt_n = load_w_tile(wk_d, 1)
            wqt_n = load_w_tile(wq_d, 1)
            for p in range(4):
                wkt_cur, wqt_cur = wkt_n, wqt_n
                if p < 2:
                    wkt_n = load_w_tile(wk_d, p + 2)
                    wqt_n = load_w_tile(wq_d, p + 2)
                for qq in range(4):
                    if qq < 3:
                        # own Q chunk qq+1, consumed by the next quarter
                        qk_proj(wq_own, q_sb, p, qq + 1, with_bias=True)
                    if p < 3:
                        if qq < 3:
                            qk_proj(wkt_cur, k_sb, p + 1, qq)
                        else:
                            qk_proj(wkt_cur, k_sb, p + 1, 3)
                            qk_proj(wqt_cur, q_sb, p + 1, 0, with_bias=True)
                    attn_pair(p, qq)
                wq_own = wqt_cur

    nc.compile()
    return nc


def make_in_maps(
    hidden_states, attention_mask, q_weight, q_bias, k_weight, k_bias, v_weight, v_bias
):
    hs = np.asarray(hidden_states, dtype=np.float32)
    am = np.asarray(attention_mask, dtype=np.float32)
    ws = {
        "q": np.asarray(q_weight, dtype=np.float32),
        "k": np.asarray(k_weight, dtype=np.float32),
        "v": np.asarray(v_weight, dtype=np.float32),
    }
    bs = {
        "q": np.asarray(q_bias, dtype=np.float32),
        "k": np.asarray(k_bias, dtype=np.float32),
        "v": np.asarray(v_bias, dtype=np.float32),
    }
    in_maps = []
    for core in range(NC):
        b, half = divmod(core, 2)
        fsl = slice(half * F, (half + 1) * F)
        in_maps.append(
            {
                "x_t": np.ascontiguousarray(
                    hs[b].T.reshape(DM, 4, 512).transpose(1, 0, 2).reshape(4 * DM, 512)
                ).astype(_bf16np),
                "wq_t": np.ascontiguousarray(
                    ws["q"][fsl, :].T.reshape(DM, 4, 128).transpose(1, 0, 2).reshape(4 * DM, 128)
                ).astype(_bf16np),
                "wk_t": np.ascontiguousarray(
                    ws["k"][fsl, :].T.reshape(DM, 4, 128).transpose(1, 0, 2).reshape(4 * DM, 128)
                ).astype(_bf16np),
                "wv_t": np.ascontiguousarray(ws["v"][fsl, :].T).astype(_bf16np),
                "bqr": np.ascontiguousarray(bs["q"][fsl]).reshape(1, F).astype(_bf16np),
                "bv": np.ascontiguousarray(bs["v"][fsl]).reshape(F, 1),
                "mask": np.ascontiguousarray(am[b, 0, 0, :].reshape(16, 128).T),
            }
        )
    return in_maps


def assemble_out(results):
    out = np.empty((4, S, DM), dtype=np.float32)
    for core in range(NC):
        b, half = divmod(core, 2)
        out[b, :, half * F : (half + 1) * F] = results[core]["out_t"].T
    return out


_NC_CACHE = []


def _run(inputs, trace=False):
    from concourse.bass_utils import run_bass_kernel_spmd

    if not _NC_CACHE:
        _NC_CACHE.append(build_nc())
    nc = _NC_CACHE[0]
    in_maps = make_in_maps(**inputs)
    res = run_bass_kernel_spmd(nc, in_maps, list(range(NC)), trace=trace)
    return assemble_out(res.results), res


def kernel(**inputs):
    out, _ = _run(inputs, trace=False)
    return out



# revision 82
# speedup vs baseline: 1.0132x; 1.0006x over previous
"""BertSelfAttention (B=4, S=2048, D=1024, H=16, hd=64) on 8 trn2 NeuronCores.

Sharding: core = 2*b + half. Each core handles batch b = core//2 and 8 of the
16 heads (feature slice half*512 .. half*512+512). Fully embarrassingly
parallel: no collectives.

Per-core kernel (bf16 operands, fp32 PSUM accumulation; measured 366.5us HW
exec, absmax rel err 9.6e-3 vs the fp32 reference, gate 2e-2):
  X^T is resident in SBUF (32KB/partition), loaded once across three DMA
  queues (first chunk per-c so the first matmul waits on 64KB only).
  Pass A: K (f-tile 0) + V projections + Q chunk 0; each pair stages its
    own remaining Q chunks one quarter ahead during attention, which
    shortens pass A and gives every quarter (incl. pair 3's) PE filler.
    Q^T, K^T in [f, s] layout (head dim on partitions); V in [s, f] layout
    with a ones column per head so the PV matmul also accumulates the
    softmax denominator in PSUM row 64. The Q bias is applied as a rank-1
    PE accumulation (bq x ones) into the projection PSUM and the K bias is
    dropped (it shifts scores by a per-query constant, which softmax
    cancels exactly), so projection staging is a plain ScalarE copy and
    attention never depends on the congested DVE queue.
  Attention per head-pair p, per q-quarter qq (512 wide):
    S^T chunks for both heads land in one [128, 1024] PSUM tile (head A in
    cols 0:512 via PE row-group 0-63, head B in cols 512:1024 via row-group
    64-127). Emission is software-pipelined in chunk pairs - the PE runs
    [PV(2t-2) PV(2t-1) QK(2t+2) QK(2t+3)] while both exp engines process
    chunks 2t/2t+1 concurrently, breaking the serial
    exp(c)->PV(c)->QK(c+1)->exp(c+1) chain. The exp work is split so
    neither engine is the wall (ScalarE alone would be 293us):
      - ScalarE chunks (10/16): exact exp activation (mask as per-partition
        bias, 1/sqrt(64) scale folded), bf16 out.
      - DVE chunks (6/16, odd slots so pairs are (ScalarE, DVE)):
        Schraudolph exp - one tensor_scalar computing
        round(23.08312*s + (184.665*mask + 16250)) into int16, which IS the
        bf16 bit pattern of ~exp(s/8 + mask). The constant-bias ratio error
        cancels exactly in the softmax ratio; the +-3% sawtooth residual
        keeps final rel err ~1e-2.
    PV accumulates ctx^T (rows 0..63) + denominator (row 64) over the 16
    k-chunks. Finalize per head: stage ctx out of PSUM on ScalarE (frees
    the PSUM bank fast, keeps DVE on exp), cross-partition copy of the den
    row + reciprocal_approx_fast (~5x cheaper than the iterative-divide
    reciprocal; the custom op needs a partition-0 input on HW), gpsimd
    partition-broadcast, DVE multiply + bias add, DMA out.
  Projection weights are prefetched a full pair ahead so the proj matmuls
  at the head of the PE queue never wait on the sync DMA queue; ~26 dummy
  matmuls during the initial DMA wait keep the HAM activity window busy so
  the first real matmuls run at 2.4GHz.
"""

import numpy as np
from ml_dtypes import bfloat16 as _bf16np

S = 2048  # sequence length
DM = 1024  # model dim
F = 512  # features per core (8 heads x 64)
HL = 8  # heads per core
HD = 64  # head dim
NC = 8  # cores

# k-chunks (of 16 per q-quarter) whose exp runs on DVE via the Schraudolph
# bit trick instead of ScalarE's exact exp. Placed at the odd slot of a
# chunk-pair so most pairs are (ScalarE, DVE) and the two engines overlap.
# Quarters where the DVE also carries finalize work get fewer trick chunks
# so the in-order DVE queue never stalls the PE's PV matmuls.
def dve_chunks(p, qq):
    return frozenset((1, 3, 5, 9, 11, 13))
# bf16 bit pattern of exp(y) ~= 184.66496*y + 16250 (128/ln2; bias tuned so
# values center in range - any constant offset cancels in the softmax ratio)
EXP_MUL = 184.66496 * 0.125  # scores arrive unscaled; fold the 1/sqrt(64)
EXP_BIAS = 16250.0


def build_nc():
    import concourse.bass as bass
    import concourse.mybir as mybir
    import concourse.tile as tile
    from concourse import bacc
    from concourse.bass import ds, ts

    f32 = mybir.dt.float32
    f32r = mybir.dt.float32r
    bf16 = mybir.dt.bfloat16
    i16 = mybir.dt.int16
    EXP = mybir.ActivationFunctionType.Exp
    PSUM = bass.MemorySpace.PSUM

    nc = bacc.Bacc("TRN2", target_bir_lowering=False, debug=False, num_devices=NC)

    x_d = nc.declare_dram_parameter("x_t", [4 * DM, 512], bf16, isOutput=False)
    bqr_d = nc.declare_dram_parameter("bqr", [1, F], bf16, isOutput=False)
    wq_d = nc.declare_dram_parameter("wq_t", [4 * DM, 128], bf16, isOutput=False)
    wk_d = nc.declare_dram_parameter("wk_t", [4 * DM, 128], bf16, isOutput=False)
    wv_d = nc.declare_dram_parameter("wv_t", [DM, F], bf16, isOutput=False)
    bv_d = nc.declare_dram_parameter("bv", [F, 1], f32, isOutput=False)
    mask_d = nc.declare_dram_parameter("mask", [128, 16], f32, isOutput=False)
    out_d = nc.declare_dram_parameter("out_t", [F, S], f32, isOutput=True)

    mm = nc.tensor.matmul

    with tile.TileContext(nc) as tc:
        with (
            tc.tile_pool(name="const", bufs=1) as const,
            tc.tile_pool(name="w", bufs=1) as wpool,
            tc.tile_pool(name="wqk", bufs=4) as wqkp,
            tc.tile_pool(name="qkv", bufs=1) as qkv,
            tc.tile_pool(name="pqkv", bufs=2, space=PSUM) as pqkv,
            tc.tile_pool(name="s_ps", bufs=2, space=PSUM) as sp,
            tc.tile_pool(name="ctxA", bufs=1, space=PSUM) as cpA,
            tc.tile_pool(name="ctxB", bufs=1, space=PSUM) as cpB,
            tc.tile_pool(name="expp", bufs=12) as ep,
            tc.tile_pool(name="stagep", bufs=14) as stp,
            tc.tile_pool(name="fin", bufs=4) as fp,
        ):
            # critical-path first: wk/wq f-tile 0 and x chunk 0 on the sync
            # queue before anything else touches DMA
            def load_w_tile(w_d, i):
                # per-c DMAs so the first proj matmul only waits for 64KB
                wt = wqkp.tile([128, 8, 128], bf16, tag="wt")
                for c in range(8):
                    nc.sync.dma_start(wt[:, c, :], w_d[ds(i * DM + c * 128, 128), :])
                return wt

            # X^T resident in SBUF for the whole kernel (32KB/partition):
            # x_sb[p, c, n, s'] = X^T[c*128+p, n*512+s']
            x_sb = qkv.tile([128, 8, 4, 512], bf16)

            def x_load(n, eng):
                # one strided DMA per chunk (descriptor issue costs ~0.7us -
                # per-c splitting jams the issuing queue), spread over
                # different engines' DMA queues so the ~15us transfers of
                # the four chunks run in parallel
                eng.dma_start(
                    x_sb[:, :, n, :],
                    x_d[ds(n * DM, DM), :].rearrange("(c p) s -> p c s", p=128),
                )

            # first chunk per-c so the very first projection matmul only
            # waits on one 64KB transfer, not the whole 512KB chunk
            for c in range(8):
                nc.gpsimd.dma_start(x_sb[:, c, 0, :], x_d[ds(c * 128, 128), :])
            # V weights next on gpsimd - the first V projection group needs
            # all 8 wv chunks ~4us after pass A starts
            wv_sb = wpool.tile([128, 8, F], bf16)
            for c in range(8):
                nc.gpsimd.dma_start(wv_sb[:, c, :], wv_d[ts(c, 128), :])
            # chunk 1 split across two queues - K(0,1) needs it ~9us after
            # pass A starts, a full-chunk transfer on one queue arrives late
            nc.scalar.dma_start(
                x_sb[:, 0:4, 1, :],
                x_d[ds(DM, 512), :].rearrange("(c p) s -> p c s", p=128),
            )
            nc.gpsimd.dma_start(
                x_sb[:, 4:8, 1, :],
                x_d[ds(DM + 512, 512), :].rearrange("(c p) s -> p c s", p=128),
            )
            x_load(2, nc.gpsimd)
            wkt = load_w_tile(wk_d, 0)
            wqt = load_w_tile(wq_d, 0)
            x_load(3, nc.sync)  # after the weight tiles on the sync queue

            # memset can't emit float32r directly; memset f32 then round-copy
            ones_f32 = const.tile([128, 128], f32)
            nc.vector.memset(ones_f32[:], 1.0)
            warm = const.tile([1, 1], f32)
            nc.scalar.activation(warm[:], ones_f32[0:1, 0:1], EXP)
            # warm the PE during the initial DMA wait: ~26 dummy matmuls keep
            # the HAM activity window busy so the first real matmuls run at
            # 2.4GHz instead of paying the 1.2GHz cold penalty
            warm_bf = const.tile([128, 512], bf16)
            nc.vector.memset(warm_bf[:], 1.0)
            warm_ps = pqkv.tile([128, 512], f32, tag="pqkv")
            for i in range(26):
                mm(
                    warm_ps[:],
                    warm_bf[:, 0:128],
                    warm_bf[:],
                    start=(i == 0),
                    stop=(i == 25),
                )

            # Q bias as a [1, 512] row (host-prepared) for the rank-1 PE
            # accumulation into the projection PSUM; K bias is dropped - it
            # only shifts scores by a per-query constant, which softmax
            # cancels exactly. A ones row is the rank-1 matmul's rhs.
            bqr_sb = const.tile([1, F], bf16)
            nc.gpsimd.dma_start(bqr_sb[:], bqr_d[:])
            ones_row = const.tile([1, 512], bf16)
            nc.vector.memset(ones_row[:], 1.0)
            bv_sb = const.tile([128, 4], f32)
            for i in range(4):
                nc.gpsimd.dma_start(bv_sb[:, i : i + 1], bv_d[ts(i, 128), :])
            mask_sb = const.tile([128, 16], f32)
            nc.gpsimd.dma_start(mask_sb[:], mask_d[:])
            # per-(k-partition, chunk) additive constant for the DVE exp trick
            c16_sb = const.tile([128, 16], f32)
            nc.vector.tensor_scalar(
                out=c16_sb[:],
                in0=mask_sb[:],
                scalar1=184.66496,
                scalar2=EXP_BIAS,
                op0=mybir.AluOpType.mult,
                op1=mybir.AluOpType.add,
            )

            # Q^T / K^T: [f, s] layout as 4 partition tiles of 128 features.
            q_sb = qkv.tile([128, 4, S], bf16)
            k_sb = qkv.tile([128, 4, S], bf16)
            # V in [k, head, d+1] layout; column 64 = 1.0 (denominator trick).
            v_sb = qkv.tile([128, 16, HL, HD + 1], bf16)
            nc.vector.tensor_copy(
                v_sb[:, :, :, HD], ones_f32[:, 0:128].rearrange("p (a b) -> p a b", a=16)
            )

            def qk_proj(wt, dst, i, n, with_bias=False):
                ps = pqkv.tile([128, 512], f32, tag="pqkv")
                for c in range(8):
                    mm(
                        ps[:],
                        wt[:, c, :],
                        x_sb[:, c, n, :],
                        start=(c == 0),
                        stop=(c == 7) and not with_bias,
                    )
                if with_bias:
                    # bias as rank-1 accumulation: ps += bq[f-tile] x ones
                    mm(
                        ps[:],
                        bqr_sb[0:1, ts(i, 128)],
                        ones_row[:],
                        start=False,
                        stop=True,
                    )
                # plain copy -> ScalarE, so the next pair's QK matmuls never
                # depend on the congested DVE queue
                nc.scalar.copy(dst[:, i, ts(n, 512)], ps[:])

            def v_proj(m, n, on_scalar=False):
                kc = n * 4 + m
                ps = pqkv.tile([128, 512], f32, tag="pqkv")
                for c in range(8):
                    mm(
                        ps[:],
                        x_sb[:, c, n, ts(m, 128)],
                        wv_sb[:, c, :],
                        start=(c == 0),
                        stop=(c == 7),
                    )
                dst = v_sb[:, kc, :, 0:HD]
                src = ps[:].rearrange("p (h d) -> p h d", h=HL)
                if on_scalar:
                    # keeps the DVE queue free for the exp trick when V is
                    # staged just-in-time inside attn(0,0)
                    nc.scalar.copy(dst, src)
                else:
                    nc.vector.tensor_copy(dst, src)

            def attn_pair(p, qq):
                hA, hB = 2 * p, 2 * p + 1
                qsl = ds(qq * 512, 512)
                ctxA = cpA.tile([HD + 1, 512], f32, tag="cA")
                ctxB = cpB.tile([HD + 1, 512], f32, tag="cB")
                def emit_qk(c):
                    sps = sp.tile([128, 1024], f32, tag="s")
                    mm(
                        sps[:, 0:512],
                        k_sb[0:64, p, ds(c * 128, 128)],
                        q_sb[0:64, p, qsl],
                        start=True,
                        stop=True,
                        tile_position=(0, 0),
                    )
                    mm(
                        sps[:, 512:1024],
                        k_sb[64:128, p, ds(c * 128, 128)],
                        q_sb[64:128, p, qsl],
                        start=True,
                        stop=True,
                        tile_position=(64, 0),
                    )
                    return sps

                dchunks = dve_chunks(p, qq)

                def emit_exp(c, sps):
                    et = ep.tile([128, 1024], bf16, tag="e")
                    if c in dchunks:
                        nc.vector.tensor_scalar(
                            out=et[:].bitcast(i16),
                            in0=sps[:],
                            scalar1=EXP_MUL,
                            scalar2=c16_sb[:, c : c + 1],
                            op0=mybir.AluOpType.mult,
                            op1=mybir.AluOpType.add,
                        )
                    else:
                        nc.scalar.activation(
                            et[:], sps[:], EXP, bias=mask_sb[:, c : c + 1], scale=0.125
                        )
                    return et

                def emit_pv(c, et):
                    mm(
                        ctxA[:],
                        v_sb[:, c, hA, :],
                        et[:, 0:512],
                        start=(c == 0),
                        stop=(c == 15),
                    )
                    mm(
                        ctxB[:],
                        v_sb[:, c, hB, :],
                        et[:, 512:1024],
                        start=(c == 0),
                        stop=(c == 15),
                    )

                # software-pipelined emission in chunk pairs: the PE queue
                # runs [PV(2t-2) PV(2t-1) QK(2t+2) QK(2t+3)] while both exp
                # engines process chunks 2t/2t+1 concurrently - the serial
                # chain exp(c)->PV(c)->QK(c+1)->exp(c+1) is broken.
                ets = {}
                for t in range(8):
                    s0 = emit_qk(2 * t)
                    s1 = emit_qk(2 * t + 1)
                    ets[2 * t] = emit_exp(2 * t, s0)
                    ets[2 * t + 1] = emit_exp(2 * t + 1, s1)
                    if t > 0:
                        emit_pv(2 * t - 2, ets.pop(2 * t - 2))
                        emit_pv(2 * t - 1, ets.pop(2 * t - 1))
                emit_pv(14, ets.pop(14))
                emit_pv(15, ets.pop(15))
                # finalize inline per head, exactly like the proven baseline
                # flow, but the reciprocal is a cheap cross-partition copy +
                # reciprocal_approx_fast (~1.3us) instead of the 3.3us
                # iterative divide, and the PSUM stage copy runs on ScalarE.
                # The very last quarter skips the stage copy (no need to
                # free PSUM fast at the end) and splits its output DMAs
                # across two queues to shorten the kernel tail.
                last = p == 3 and qq == 3
                for h, ctx in ((hA, ctxA), (hB, ctxB)):
                    if last:
                        stage = ctx
                    else:
                        stage = stp.tile([HD + 1, 512], f32, tag="stage")
                        nc.scalar.copy(stage[:], ctx[:])
                    den = fp.tile([1, 512], f32, tag="den")
                    nc.vector.tensor_copy(den[:], stage[HD : HD + 1, :])
                    recip = fp.tile([1, 512], f32, tag="recip")
                    nc.vector.reciprocal_approx_fast(recip[:], den[:])
                    bcast = fp.tile([64, 512], f32, tag="bcast")
                    nc.gpsimd.partition_broadcast(bcast[:], recip[:])
                    out_sb = fp.tile([64, 512], f32, tag="out")
                    nc.vector.tensor_mul(out_sb[:], stage[0:HD, :], bcast[:])
                    rp = (h % 2) * 64
                    nc.vector.tensor_scalar_add(
                        out_sb[:], out_sb[:], bv_sb[rp : rp + 64, h // 2 : h // 2 + 1]
                    )
                    if last:
                        nc.sync.dma_start(
                            out_d[ds(h * 64, 64), ds(qq * 512, 256)], out_sb[:, 0:256]
                        )
                        nc.gpsimd.dma_start(
                            out_d[ds(h * 64, 64), ds(qq * 512 + 256, 256)],
                            out_sb[:, 256:512],
                        )
                    else:
                        nc.sync.dma_start(
                            out_d[ds(h * 64, 64), ds(qq * 512, 512)], out_sb[:]
                        )

            # ---- pass A: K, V projections (f-tile 0) + Q chunk 0 only;
            # each pair stages its own remaining Q chunks one quarter ahead,
            # which shortens pass A and gives pair 3 PE filler work ----
            for n in range(4):
                qk_proj(wkt, k_sb, 0, n)
                for m in range(4):
                    v_proj(m, n)
            qk_proj(wqt, q_sb, 0, 0, with_bias=True)
            wq_own = wqt

            # ---- attention pair p overlapped with projections for p+1:
            # per quarter, one own-pair Q group (needed next quarter) plus
            # one next-pair group (K first - attention qq=0 of the next pair
            # reads ALL K n-chunks; its Q chunk 0 last). Weights prefetched
            # a full pair ahead so proj matmuls at the head of the PE queue
            # never wait on the (congested) sync DMA queue. ----
            wkt_n = load_w_tile(wk_d, 1)
            wqt_n = load_w_tile(wq_d, 1)
            for p in range(4):
                wkt_cur, wqt_cur = wkt_n, wqt_n
                if p < 2:
                    wkt_n = load_w_tile(wk_d, p + 2)
                    wqt_n = load_w_tile(wq_d, p + 2)
                for qq in range(4):
                    if qq < 3:
                        # own Q chunk qq+1, consumed by the next quarter
                        qk_proj(wq_own, q_sb, p, qq + 1, with_bias=True)
                    if p < 3:
                        if qq < 3:
                            qk_proj(wkt_cur, k_sb, p + 1, qq)
                        else:
                            qk_proj(wkt_cur, k_sb, p + 1, 3)
                            qk_proj(wqt_cur, q_sb, p + 1, 0, with_bias=True)
                    attn_pair(p, qq)
                wq_own = wqt_cur

    nc.compile()
    return nc


def make_in_maps(
    hidden_states, attention_mask, q_weight, q_bias, k_weight, k_bias, v_weight, v_bias
):
    hs = np.asarray(hidden_states, dtype=np.float32)
    am = np.asarray(attention_mask, dtype=np.float32)
    ws = {
        "q": np.asarray(q_weight, dtype=np.float32),
        "k": np.asarray(k_weight, dtype=np.float32),
        "v": np.asarray(v_weight, dtype=np.float32),
    }
    bs = {
        "q": np.asarray(q_bias, dtype=np.float32),
        "k": np.asarray(k_bias, dtype=np.float32),
        "v": np.asarray(v_bias, dtype=np.float32),
    }
    in_maps = []
    for core in range(NC):
        b, half = divmod(core, 2)
        fsl = slice(half * F, (half + 1) * F)
        in_maps.append(
            {
                "x_t": np.ascontiguousarray(
                    hs[b].T.reshape(DM, 4, 512).transpose(1, 0, 2).reshape(4 * DM, 512)
                ).astype(_bf16np),
                "wq_t": np.ascontiguousarray(
                    ws["q"][fsl, :].T.reshape(DM, 4, 128).transpose(1, 0, 2).reshape(4 * DM, 128)
                ).astype(_bf16np),
                "wk_t": np.ascontiguousarray(
                    ws["k"][fsl, :].T.reshape(DM, 4, 128).transpose(1, 0, 2).reshape(4 * DM, 128)
                ).astype(_bf16np),
                "wv_t": np.ascontiguousarray(ws["v"][fsl, :].T).astype(_bf16np),
                "bqr": np.ascontiguousarray(bs["q"][fsl]).reshape(1, F).astype(_bf16np),
                "bv": np.ascontiguousarray(bs["v"][fsl]).reshape(F, 1),
                "mask": np.ascontiguousarray(am[b, 0, 0, :].reshape(16, 128).T),
            }
        )
    return in_maps


def assemble_out(results):
    out = np.empty((4, S, DM), dtype=np.float32)
    for core in range(NC):
        b, half = divmod(core, 2)
        out[b, :, half * F : (half + 1) * F] = results[core]["out_t"].T
    return out


_NC_CACHE = []


def _run(inputs, trace=False):
    from concourse.bass_utils import run_bass_kernel_spmd

    if not _NC_CACHE:
        _NC_CACHE.append(build_nc())
    nc = _NC_CACHE[0]
    in_maps = make_in_maps(**inputs)
    res = run_bass_kernel_spmd(nc, in_maps, list(range(NC)), trace=trace)
    return assemble_out(res.results), res


def kernel(**inputs):
    out, _ = _run(inputs, trace=False)
    return out

